# revision 35
# baseline (speedup 1.0000x reference)
"""Trainium2 Bass kernel for nn_CSAModule_47768626266174.

Mathematical structure of the reference:

    S    = softmax(attn, axis=-1)                # [C, T, T]
    out  = base + sigma * einsum('bft,ct->bcf', inputs, S.mean(axis=-1))
    base = inputs.mean(-1)[:, None, :]

``S.mean(axis=-1)`` averages over the *same* axis the softmax
normalizes, so it is exactly 1/T for every (c, t) — independent of the
attention contents, the conv weights, and the labels.  Hence

    out[b, c, f] = (1 + sigma) * mean_t inputs[b, f, t]

for every class c: the kernel only needs to read ``inputs`` once,
reduce over T, scale by (1+sigma)/T, and broadcast over the class dim.

Sharding: data-parallel over batch B — each of the 8 cores reduces its
8-item chunk; no collectives.  Output chunks are concatenated on host.

Per-core dataflow.  The critical path is the serialized DMA stream then
the tail chain of the LAST-arriving chunk: dma-sem (900 ns) -> reduce ->
matmul -> PSUM->SBUF copy -> store launch (HWDGE 625 + DGE 650) ->
transfer -> dma-sem (900 ns).  Key design points:

  * SP's 5-instruction register preamble is skipped and the loads are
    emitted before the Block entry branch: the first load DMA issues at
    t=0.  The Bass start barrier, its dead const-tile memsets, and the
    Block-exit all-engine barrier are also skipped; SP's final waits on
    the two store semaphores keep the program alive until y is in HBM.
  * Items 0/1 arrive through ONE casting SWDGE load (fp32 -> bf16,
    descriptors generated on the otherwise-idle GPSIMD engine): the DMA
    engines move half the bytes for those items, shortening the stream
    ~720 ns.  Sigma also loads via SWDGE, generated FIRST so the
    s1 -> scale_col -> ident chain resolves early.  More cast pairs do
    NOT help: DVE's reduce throughput (1.04 ns/element regardless of
    dtype) becomes the wall and the copies pile up behind it.
  * Items 2-6 and item 7's two halves load via plain HWDGE from SP
    (7 loads; HWDGE costs 625 ns per DMA, so more/smaller loads would
    stall the stream behind descriptor generation).
  * DVE reduces items 2,3,4, then 0,1 (bf16), then 5, then 7's halves —
    ordered by semaphore time so the final reduces run semaphore-paced.
    Item 6 reduces on ACT via activation+accumulate.
  * Items 0,1,7 produce bf16 sums and their matmuls use a bf16 scaled
    identity: 1 PE cycle/row instead of fp32's 4 (53 vs 213 ns).
    Measured whole-output rel-err ~1.5e-3 vs a 2e-2 budget.
  * PE uses STANDALONE wait_ge between matmuls: the busy sequencer pins
    pe_busy_start so the p-state ramps to full speed instead of
    restarting cold (607 ns/matmul) every time.  Item 7's two
    accumulating matmuls sandwich item 6's (different PSUM banks).
  * Copies: c2,c3,c4,c0,c1,c5 on ACT (interleaved with item 6's
    reduce); c6,c7 on DVE (free after its last reduce; GPSIMD may not
    touch PSUM).  Two HWDGE stores: y[0:5] (early) and y[5:8] (the
    only store on the critical tail).
  * SWDGE prepare/trigger stores would launch ~1.2 us faster after the
    last copy, but this walrus build cannot encode InstTriggerDma
    ("ISA wrong length"), so the HWDGE store path it is.
"""

from contextlib import ExitStack

import numpy as np

B, F, T, C = 64, 128, 512, 10
N_CORES = 8
BPC = B // N_CORES  # batch items per core
H = T // 2

_NC_CACHE = None


def _build_bass():
    """Build the per-core Bass module (SPMD: same program on all cores)."""
    global _NC_CACHE
    if _NC_CACHE is not None:
        return _NC_CACHE

    import concourse.bass as bass
    import concourse.mybir as mybir

    fp32 = mybir.dt.float32
    bf16 = mybir.dt.bfloat16

    _orig_memset = bass.BassEitherVectorEngine.memset

    def _memset_skip_dead_consts(self, ap, constant):
        tensor = getattr(ap, "tensor", None)
        if tensor is not None and getattr(tensor, "name", "").startswith(
            "const-"
        ):
            return None
        return _orig_memset(self, ap, constant)

    _orig_barrier = bass.Bass.all_engine_barrier

    def _skip_barrier(self, *, sem_only: bool = False):
        return None

    _orig_preamble = bass.BassEngine.preamble

    def _preamble_skip_sp(self):
        if self.engine == mybir.EngineType.SP:
            return None
        return _orig_preamble(self)

    bass.BassEitherVectorEngine.memset = _memset_skip_dead_consts
    bass.Bass.all_engine_barrier = _skip_barrier
    bass.BassEngine.preamble = _preamble_skip_sp
    try:
        nc = bass.Bass()

        x = nc.dram_tensor("x", [BPC, F, T], fp32, kind="ExternalInput")
        sig = nc.dram_tensor("sig", [1, 1], fp32, kind="ExternalInput")
        y = nc.dram_tensor("y", [BPC, C, F], fp32, kind="ExternalOutput")

        with ExitStack() as ctx:
            e = ctx.enter_context
            # xt holds items 2..7 (fp32): item b at cols (b-2)*T.
            # Items 0/1 land as bf16 via one casting SWDGE load.
            xt = e(nc.sbuf_tensor("xt", [128, 6 * T], fp32))
            xt16 = e(nc.sbuf_tensor("xt16", [128, 2 * T], bf16))
            # sums: fp32 cols for items 2-6; bf16 cols 0,1=items 0,1 and
            # 6,7=item 7's halves.
            sums = e(nc.sbuf_tensor("sums", [128, 8], fp32))
            sums16 = e(nc.sbuf_tensor("sums16", [128, 8], bf16))
            ident_s = e(nc.sbuf_tensor("ident_s", [128, 128], fp32))
            ident16 = e(nc.sbuf_tensor("ident16", [128, 128], bf16))
            sg = e(nc.sbuf_tensor("sg", [1, 1], fp32))
            s1 = e(nc.sbuf_tensor("s1", [1, 1], fp32))
            ones_row = e(nc.sbuf_tensor("ones_row", [1, 128], fp32))
            scale_col = e(nc.sbuf_tensor("scale_col", [128, 1], fp32))
            yt = e(nc.sbuf_tensor("yt", [C, BPC * F], fp32))
            dump = e(nc.sbuf_tensor("dump", [128, T], fp32))
            # psc is allocated and immediately freed: its bank is reused by
            # pts[0].  Safe because mm0 (the first pts[0] write) waits for
            # ident_s, which waits for the scale_col copy — psc's last read.
            psc_cm = nc.psum_tensor("psc", [128, 1], fp32)
            psc = psc_cm.__enter__()
            psc_cm.__exit__(None, None, None)
            pts = [
                e(nc.psum_tensor(f"pt{b}", [C, 128], fp32))
                for b in range(BPC)
            ]

            c01_sem = e(nc.semaphore("xc01"))
            # HWDGE loads (SP): (item, xt-t0, t1, sem, engine); item b at
            # xt cols (b-2)*T.
            loads = []
            for b in range(2, 6):
                loads.append(
                    (b, (b - 2) * T, (b - 1) * T, e(nc.semaphore(f"x{b}")), "dve")
                )
            loads.append((6, 4 * T, 5 * T, e(nc.semaphore("x6")), "act"))
            x7a_sem = e(nc.semaphore("x7a"))
            x7b_sem = e(nc.semaphore("x7b"))
            loads.append((7, 5 * T, 5 * T + H, x7a_sem, "dve16"))
            loads.append((7, 5 * T + H, 6 * T, x7b_sem, "dve16"))

            sig_sem = e(nc.semaphore("sig_sem"))
            s1_sem = e(nc.semaphore("s1_sem"))
            dve_sem = e(nc.semaphore("dve_sem"))
            pool_sem = e(nc.semaphore("pool_sem"))
            pe_sem = e(nc.semaphore("pe_sem"))
            act_sem = e(nc.semaphore("act_sem"))
            act_red_sem = e(nc.semaphore("act_red_sem"))
            c67_sem = e(nc.semaphore("c67_sem"))
            ca_sem = e(nc.semaphore("ca_sem"))
            sa_sem = e(nc.semaphore("sa_sem"))
            sb_sem = e(nc.semaphore("sb_sem"))
            sc_sem = e(nc.semaphore("sc_sem"))

            # DVE reduce order (by semaphore time): r2, r3, r4, then the
            # cast pair r0/r1, r5, then item 7's halves.  Reduce spec:
            # (key, src, s0, s1col, out16?, waitsem).  sums16 cols: 0,1 =
            # items 0,1; 2,3 = item 7's halves.  sums (fp32) cols 2..5 =
            # items 2..5 (item 6 accumulates into col 6 on ACT).
            dve_reduces = [
                (2, xt, 0 * T, 1 * T, False, None),
                (3, xt, 1 * T, 2 * T, False, None),
                (4, xt, 2 * T, 3 * T, False, None),
                ("01a", xt16, 0 * T, 1 * T, True, c01_sem),
                ("01b", xt16, 1 * T, 2 * T, True, c01_sem),
                (5, xt, 3 * T, 4 * T, False, None),
                ("7a", xt, 5 * T, 5 * T + H, True, x7a_sem),
                ("7b", xt, 5 * T + H, 6 * T, True, x7b_sem),
            ]
            load_sems = {b: sem for b, t0, t1, sem, eng in loads}
            red_out16 = {"01a": 0, "01b": 1, "7a": 2, "7b": 3}
            red_out32 = {2: 2, 3: 3, 4: 4, 5: 5}
            dve_ms = {}
            n = 1
            for entry in dve_reduces:
                n += 1
                dve_ms[entry[0]] = n
            # pe_sem: 1 = psc, then the matmuls in PE program order.
            # ("f32", item, sums-col) / ("b16", item, sums16-col, start,
            # stop).  Item 7 accumulates in pts[7]; mm6 interleaves
            # between mm7a and mm7b (different PSUM banks).
            mm_order = [
                ("f32", 2, 2, True, True),
                ("f32", 3, 3, True, True),
                ("f32", 4, 4, True, True),
                ("b16", 0, 0, True, True),
                ("b16", 1, 1, True, True),
                ("f32", 5, 5, True, True),
                ("b16", 7, 2, True, False),
                ("mm6", 6, 6, True, True),
                ("b16", 7, 3, False, True),
            ]
            mm_ms = {}
            for i, ent in enumerate(mm_order):
                mm_ms[(ent[0], ent[2])] = i + 2
            MM_BY_ITEM = {
                0: mm_ms[("b16", 0)],
                1: mm_ms[("b16", 1)],
                2: mm_ms[("f32", 2)],
                3: mm_ms[("f32", 3)],
                4: mm_ms[("f32", 4)],
                5: mm_ms[("f32", 5)],
                6: mm_ms[("mm6", 6)],
                7: mm_ms[("b16", 3)],
            }
            MM7_DONE = MM_BY_ITEM[7]

            # Loads issue in the main body, before the Block's entry
            # branch: SP's first DMA starts at t=0 instead of t=50.
            for b, t0, t1, sem, eng in loads:
                nc.sync.dma_start(
                    xt[:, t0:t1], x[b, :, t0 - (b - 2) * T : t1 - (b - 2) * T]
                ).then_inc(sem, 16)

            block = e(nc.Block())

            @block.sync
            def _(sync):
                # Store A: items 0-4 (c0..c4 bump ca_sem).
                sync.dma_start(
                    y[0:5, :, :].rearrange("b c f -> c b f"),
                    yt[:, 0 : 5 * F].rearrange("c (b f) -> c b f", f=F),
                )._wait_ge(ca_sem, 5).then_inc(sa_sem, 16)
                # Store B: items 5-7 (c5/c6/c7 bump c67_sem).
                sync.dma_start(
                    y[5:8, :, :].rearrange("b c f -> c b f"),
                    yt[:, 5 * F : 8 * F].rearrange("c (b f) -> c b f", f=F),
                )._wait_ge(c67_sem, 3).then_inc(sb_sem, 16)
                sync.wait_ge(sa_sem, 16)
                sync.wait_ge(sb_sem, 16)

            @block.vector
            def _(vector):
                vector.memset(ones_row[:, :], 1.0).then_inc(dve_sem, 1)
                first = True
                for key, src_t, a0, a1, out16, wsem in dve_reduces:
                    if wsem is None:
                        wsem = load_sems[key]
                    if out16:
                        with nc.allow_low_precision(
                            "bf16 sums feed bf16 matmuls; ~0.4% rounding"
                        ):
                            col = red_out16[key]
                            red = vector.reduce_sum(
                                out=sums16[:, col : col + 1],
                                in_=src_t[:, a0:a1],
                                axis=mybir.AxisListType.X,
                            )
                    else:
                        col = red_out32[key]
                        red = vector.reduce_sum(
                            out=sums[:, col : col + 1],
                            in_=src_t[:, a0:a1],
                            axis=mybir.AxisListType.X,
                        )
                    red._wait_ge(wsem, 16).then_inc(dve_sem, 1)
                    if first:
                        first = False
                        # s1 = (1+sigma)/T right after the first reduce.
                        vector.tensor_scalar(
                            out=s1[:, :],
                            in0=sg[:, :],
                            scalar1=1.0 / T,
                            scalar2=1.0 / T,
                            op0=mybir.AluOpType.mult,
                            op1=mybir.AluOpType.add,
                        )._wait_ge(sig_sem, 16).then_inc(s1_sem, 1)
                # Items 6 and 7's PSUM -> SBUF copies (GPSIMD may not
                # touch PSUM; DVE is free after its last reduce; ACT is
                # backlogged with item 6's accum-reduce and c1/c5).
                vector.tensor_copy(
                    yt[:, 6 * F : 7 * F], pts[6][:, :]
                )._wait_ge(pe_sem, MM_BY_ITEM[6]).then_inc(c67_sem, 1)
                vector.tensor_copy(
                    yt[:, 7 * F : 8 * F], pts[7][:, :]
                )._wait_ge(pe_sem, MM7_DONE).then_inc(c67_sem, 1)

            @block.gpsimd
            def _(gpsimd):
                # SWDGE loads: sigma first (tiny; its early semaphore
                # unblocks the s1 -> ident chain), then the casting pair
                # load of items 0/1 (fp32 -> bf16: half the DMA bytes).
                gpsimd.dma_start(sg[:, :], sig[:, :]).then_inc(sig_sem, 16)
                gpsimd.dma_start(
                    xt16[:, :].rearrange("p (b t) -> p b t", b=2),
                    x[0:2, :, :].rearrange("b p t -> p b t"),
                ).then_inc(c01_sem, 16)
                # ident_s = diag((1+sigma)/T): select between a broadcast
                # of scale_col and 0.0.
                gpsimd.affine_select(
                    out=ident_s[:, :],
                    in_=scale_col[:, :].broadcast_to((128, 128)),
                    compare_op=mybir.AluOpType.is_equal,
                    fill=0.0,
                    base=0,
                    pattern=[[-1, 128]],
                    channel_multiplier=1,
                )._wait_ge(act_sem, 1).then_inc(pool_sem, 1)
                gpsimd.tensor_copy(ident16[:, :], ident_s[:, :])

            @block.scalar
            def _(scalar):
                # scale_col = (1+sigma)/T on all partitions, via psc.
                scalar.activation(
                    out=scale_col[:, :],
                    in_=psc[:, :],
                    func=mybir.ActivationFunctionType.Copy,
                )._wait_ge(pe_sem, 1).then_inc(act_sem, 1)
                # Copies c2, c3, c4 (early, reduce-paced), then c0, then
                # item 6's accum-reduce, then c1 and c5 as their matmuls
                # land.  c6/c7 run on DVE.
                for bidx in (2, 3, 4):
                    scalar.activation(
                        out=yt[:, bidx * F : (bidx + 1) * F],
                        in_=pts[bidx][:, :],
                        func=mybir.ActivationFunctionType.Copy,
                    )._wait_ge(pe_sem, MM_BY_ITEM[bidx]).then_inc(ca_sem, 1)
                scalar.activation(
                    out=yt[:, 0 * F : 1 * F],
                    in_=pts[0][:, :],
                    func=mybir.ActivationFunctionType.Copy,
                )._wait_ge(pe_sem, MM_BY_ITEM[0]).then_inc(ca_sem, 1)
                # Item 6's reduce: activation+accumulate into sums col 6.
                scalar.activation(
                    out=dump[:, 0:T],
                    in_=xt[:, 4 * T : 5 * T],
                    func=mybir.ActivationFunctionType.Copy,
                    accum_out=sums[:, 6:7],
                )._wait_ge(load_sems[6], 16).then_inc(act_red_sem, 1)
                scalar.activation(
                    out=yt[:, 1 * F : 2 * F],
                    in_=pts[1][:, :],
                    func=mybir.ActivationFunctionType.Copy,
                )._wait_ge(pe_sem, MM_BY_ITEM[1]).then_inc(ca_sem, 1)
                scalar.activation(
                    out=yt[:, 5 * F : 6 * F],
                    in_=pts[5][:, :],
                    func=mybir.ActivationFunctionType.Copy,
                )._wait_ge(pe_sem, MM_BY_ITEM[5]).then_inc(c67_sem, 1)

            @block.tensor
            def _(tensor):
                # Standalone waits keep PE.SEQ occupied between matmuls,
                # pinning pe_busy_start so the p-state ramps to full speed.
                # psc[p, 0] = (1+sigma)/T on every partition (K=1 matmul).
                tensor.wait_ge(dve_sem, 1)  # ones_row
                tensor.wait_ge(s1_sem, 1)
                tensor.matmul(
                    psc[:, :], ones_row[:, :], s1[:, :], start=True, stop=True
                ).then_inc(pe_sem, 1)
                tensor.wait_ge(pool_sem, 1)  # ident_s ready
                red_key = {
                    ("b16", 0): "01a",
                    ("b16", 1): "01b",
                    ("b16", 2): "7a",
                    ("b16", 3): "7b",
                    ("f32", 2): 2,
                    ("f32", 3): 3,
                    ("f32", 4): 4,
                    ("f32", 5): 5,
                }
                for kind, b, col, is_start, is_stop in mm_order:
                    if kind == "mm6":
                        tensor.wait_ge(act_red_sem, 1)
                        lhsT = sums[:, 6:7].broadcast_to((128, C))
                        rhs = ident_s[:, :]
                    else:
                        tensor.wait_ge(dve_sem, dve_ms[red_key[(kind, col)]])
                        if kind == "b16":
                            lhsT = sums16[:, col : col + 1].broadcast_to(
                                (128, C)
                            )
                            rhs = ident16[:, :]
                        else:
                            lhsT = sums[:, col : col + 1].broadcast_to(
                                (128, C)
                            )
                            rhs = ident_s[:, :]
                    tensor.matmul(
                        pts[b][:, :],
                        lhsT,
                        rhs,
                        start=is_start,
                        stop=is_stop,
                    ).then_inc(pe_sem, 1)

    finally:
        bass.BassEitherVectorEngine.memset = _orig_memset
        bass.Bass.all_engine_barrier = _orig_barrier
        bass.BassEngine.preamble = _orig_preamble

    _NC_CACHE = nc
    return nc


def run_spmd(inputs_arr: np.ndarray, sigma_arr: np.ndarray, trace: bool = False):
    """Shard over batch, run on 8 cores, gather. Returns (out, results_obj)."""
    from concourse import bass_utils

    nc = _build_bass()

    x_full = np.ascontiguousarray(np.asarray(inputs_arr, dtype=np.float32))
    assert x_full.shape == (B, F, T), x_full.shape
    sig = np.asarray(sigma_arr, dtype=np.float32).reshape(1, 1)

    in_maps = [
        {"x": x_full[k * BPC : (k + 1) * BPC], "sig": sig} for k in range(N_CORES)
    ]
    res = bass_utils.run_bass_kernel_spmd(
        nc, in_maps, core_ids=list(range(N_CORES)), trace=trace
    )
    out = np.concatenate([r["y"] for r in res.results], axis=0)
    return out, res


def kernel(**inputs) -> np.ndarray:
    out, _ = run_spmd(inputs["inputs"], inputs["sigma"])
    return out


# revision 36
# speedup vs baseline: 1.0109x; 1.0109x over previous
"""Trainium2 Bass kernel for nn_CSAModule_47768626266174 — v9.

v7 + items 0/1 arrive through ONE casting SWDGE load (fp32->bf16,
descriptors generated on idle GPSIMD; the DMA engines move half the
bytes, shortening the stream ~720 ns), sigma through a SWDGE load
generated first.

Mathematical structure of the reference:

    S    = softmax(attn, axis=-1)                # [C, T, T]
    out  = base + sigma * einsum('bft,ct->bcf', inputs, S.mean(axis=-1))
    base = inputs.mean(-1)[:, None, :]

``S.mean(axis=-1)`` averages over the *same* axis the softmax normalizes,
so it is exactly 1/T for every (c, t) — independent of the attention
contents, the conv weights, and the labels.  Hence

    out[b, c, f] = (1 + sigma) * mean_t inputs[b, f, t]

for every class c, so the kernel only needs to read ``inputs`` once,
reduce over T, scale by (1 + sigma)/T, and broadcast over the class dim.

Sharding: data-parallel over batch B — each of the 8 cores reduces its
8-item chunk; no collectives.  Output chunks are concatenated on host.

Per-core dataflow.  The critical path is the serialized DMA data stream
(~2.1 MB at 360 GB/s), then the tail chain of the LAST-arriving chunk:
dma-sem (900) -> reduce -> matmul -> PSUM->SBUF copy -> store launch
(HWDGE 625 + DGE 650) -> transfer -> dma-sem (900).  Design notes:

  * SP's 5-instruction register preamble is skipped (-250 ns stream
    start; SP only issues DMAs / waits, which never read those regs).
  * 9 load DMAs (HWDGE is 625 ns per DMA; a 10th would stall the
    stream): items 0-6 whole, item 7 in halves so the last chunk's
    reduce is half-length.  Each load has its own semaphore (dynamic
    queues complete out of order).
  * DVE reduces items 0-5 and 7a/7b; item 6 reduces on ACT via
    activation+accumulate, so DVE is FREE exactly when 7a/7b's
    semaphores fire and the last reduces run semaphore-paced.
  * Item 7's sums are written in bf16 and its two matmuls use a bf16
    scaled-identity: 1 PE cycle/row instead of fp32's 4 (53 vs 213 ns).
    ~0.4% rounding on 1/8 of the output, vs a 2e-2 rel-err budget.
  * PE matmul order: items 0-5, then 7a(start), 6, 7b(stop).  mm7a's
    input is ready before mm6's (ACT's accum-read chain is slow) and
    mm7b's reduce lands last, so this keeps PE busy without delaying
    mm7b.  pts[6]/pts[7] are separate PSUM banks so the interleaved
    accumulation groups don't interact.  PE uses STANDALONE wait_ge
    (not waits attached to the matmul): the busy sequencer pins
    pe_busy_start so the PE p-state ramps to full speed (213 ns/mm)
    instead of restarting cold (607 ns/mm) at every matmul.
  * Copies: items 0-5 and 6 on ACT into yt; item 7 on DVE (GPSIMD may
    not touch PSUM; DVE is free after its last reduce).
  * Three HWDGE stores from SP: items 0-3 (after c3), 4-5 (after c5),
    6-7 (after c6+c7, the only store on the critical tail).  SWDGE
    prepare/trigger would launch ~1.2 us faster after the data is
    ready, but this walrus build cannot encode InstTriggerDma
    ("ISA wrong length"), so the HWDGE path it is.
  * The Bass start barrier, its dead const-tile memsets, and the
    Block-exit all-engine barrier are skipped; SP's final waits on the
    three store semaphores keep the program alive until y is in HBM.
"""

from contextlib import ExitStack

import numpy as np

B, F, T, C = 64, 128, 512, 10
N_CORES = 8
BPC = B // N_CORES  # batch items per core
H = T // 2

_NC_CACHE = None


def _build_bass():
    """Build the per-core Bass module (SPMD: same program on all cores)."""
    global _NC_CACHE
    if _NC_CACHE is not None:
        return _NC_CACHE

    import concourse.bass as bass
    import concourse.mybir as mybir

    fp32 = mybir.dt.float32
    bf16 = mybir.dt.bfloat16

    _orig_memset = bass.BassEitherVectorEngine.memset

    def _memset_skip_dead_consts(self, ap, constant):
        tensor = getattr(ap, "tensor", None)
        if tensor is not None and getattr(tensor, "name", "").startswith(
            "const-"
        ):
            return None
        return _orig_memset(self, ap, constant)

    _orig_barrier = bass.Bass.all_engine_barrier

    def _skip_barrier(self, *, sem_only: bool = False):
        return None

    _orig_preamble = bass.BassEngine.preamble

    def _preamble_skip_sp(self):
        if self.engine == mybir.EngineType.SP:
            return None
        return _orig_preamble(self)

    bass.BassEitherVectorEngine.memset = _memset_skip_dead_consts
    bass.Bass.all_engine_barrier = _skip_barrier
    bass.BassEngine.preamble = _preamble_skip_sp
    try:
        nc = bass.Bass()

        x = nc.dram_tensor("x", [BPC, F, T], fp32, kind="ExternalInput")
        sig = nc.dram_tensor("sig", [1, 1], fp32, kind="ExternalInput")
        y = nc.dram_tensor("y", [BPC, C, F], fp32, kind="ExternalOutput")

        with ExitStack() as ctx:
            e = ctx.enter_context
            # xt holds items 2..7 (fp32): item b at cols (b-2)*T.
            # Items 0/1 land as bf16 via one casting SWDGE load.
            xt = e(nc.sbuf_tensor("xt", [128, 6 * T], fp32))
            xt16 = e(nc.sbuf_tensor("xt16", [128, 2 * T], bf16))
            # sums: fp32 cols for items 2-6; bf16 cols 0,1=items 0,1 and
            # 6,7=item 7's halves.
            sums = e(nc.sbuf_tensor("sums", [128, 8], fp32))
            sums16 = e(nc.sbuf_tensor("sums16", [128, 8], bf16))
            ident_s = e(nc.sbuf_tensor("ident_s", [128, 128], fp32))
            ident16 = e(nc.sbuf_tensor("ident16", [128, 128], bf16))
            sg = e(nc.sbuf_tensor("sg", [1, 1], fp32))
            s1 = e(nc.sbuf_tensor("s1", [1, 1], fp32))
            ones_row = e(nc.sbuf_tensor("ones_row", [1, 128], fp32))
            scale_col = e(nc.sbuf_tensor("scale_col", [128, 1], fp32))
            yt = e(nc.sbuf_tensor("yt", [C, BPC * F], fp32))
            dump = e(nc.sbuf_tensor("dump", [128, T], fp32))
            # psc is allocated and immediately freed: its bank is reused by
            # pts[0].  Safe because mm0 (the first pts[0] write) waits for
            # ident_s, which waits for the scale_col copy — psc's last read.
            psc_cm = nc.psum_tensor("psc", [128, 1], fp32)
            psc = psc_cm.__enter__()
            psc_cm.__exit__(None, None, None)
            pts = [
                e(nc.psum_tensor(f"pt{b}", [C, 128], fp32))
                for b in range(BPC)
            ]

            c01_sem = e(nc.semaphore("xc01"))
            # HWDGE loads (SP): (item, xt-t0, t1, sem, engine); item b at
            # xt cols (b-2)*T.
            loads = []
            for b in range(2, 6):
                loads.append(
                    (b, (b - 2) * T, (b - 1) * T, e(nc.semaphore(f"x{b}")), "dve")
                )
            loads.append((6, 4 * T, 5 * T, e(nc.semaphore("x6")), "act"))
            x7a_sem = e(nc.semaphore("x7a"))
            x7b_sem = e(nc.semaphore("x7b"))
            loads.append((7, 5 * T, 5 * T + H, x7a_sem, "dve16"))
            loads.append((7, 5 * T + H, 6 * T, x7b_sem, "dve16"))

            sig_sem = e(nc.semaphore("sig_sem"))
            s1_sem = e(nc.semaphore("s1_sem"))
            dve_sem = e(nc.semaphore("dve_sem"))
            pool_sem = e(nc.semaphore("pool_sem"))
            pe_sem = e(nc.semaphore("pe_sem"))
            act_sem = e(nc.semaphore("act_sem"))
            act_red_sem = e(nc.semaphore("act_red_sem"))
            c67_sem = e(nc.semaphore("c67_sem"))
            ca_sem = e(nc.semaphore("ca_sem"))
            sa_sem = e(nc.semaphore("sa_sem"))
            sb_sem = e(nc.semaphore("sb_sem"))
            sc_sem = e(nc.semaphore("sc_sem"))

            # DVE reduce order (by semaphore time): r2, r3, r4, then the
            # cast pair r0/r1, r5, then item 7's halves.  Reduce spec:
            # (key, src, s0, s1col, out16?, waitsem).  sums16 cols: 0,1 =
            # items 0,1; 2,3 = item 7's halves.  sums (fp32) cols 2..5 =
            # items 2..5 (item 6 accumulates into col 6 on ACT).
            dve_reduces = [
                (2, xt, 0 * T, 1 * T, False, None),
                (3, xt, 1 * T, 2 * T, False, None),
                (4, xt, 2 * T, 3 * T, False, None),
                ("01a", xt16, 0 * T, 1 * T, True, c01_sem),
                ("01b", xt16, 1 * T, 2 * T, True, c01_sem),
                (5, xt, 3 * T, 4 * T, False, None),
                ("7a", xt, 5 * T, 5 * T + H, True, x7a_sem),
                ("7b", xt, 5 * T + H, 6 * T, True, x7b_sem),
            ]
            load_sems = {b: sem for b, t0, t1, sem, eng in loads}
            red_out16 = {"01a": 0, "01b": 1, "7a": 2, "7b": 3}
            red_out32 = {2: 2, 3: 3, 4: 4, 5: 5}
            dve_ms = {}
            n = 1
            for entry in dve_reduces:
                n += 1
                dve_ms[entry[0]] = n
            # pe_sem: 1 = psc, then the matmuls in PE program order.
            # ("f32", item, sums-col) / ("b16", item, sums16-col, start,
            # stop).  Item 7 accumulates in pts[7]; mm6 interleaves
            # between mm7a and mm7b (different PSUM banks).
            mm_order = [
                ("f32", 2, 2, True, True),
                ("f32", 3, 3, True, True),
                ("f32", 4, 4, True, True),
                ("b16", 0, 0, True, True),
                ("b16", 1, 1, True, True),
                ("f32", 5, 5, True, True),
                ("b16", 7, 2, True, False),
                ("mm6", 6, 6, True, True),
                ("b16", 7, 3, False, True),
            ]
            mm_ms = {}
            for i, ent in enumerate(mm_order):
                mm_ms[(ent[0], ent[2])] = i + 2
            MM_BY_ITEM = {
                0: mm_ms[("b16", 0)],
                1: mm_ms[("b16", 1)],
                2: mm_ms[("f32", 2)],
                3: mm_ms[("f32", 3)],
                4: mm_ms[("f32", 4)],
                5: mm_ms[("f32", 5)],
                6: mm_ms[("mm6", 6)],
                7: mm_ms[("b16", 3)],
            }
            MM7_DONE = MM_BY_ITEM[7]

            # Loads issue in the main body, before the Block's entry
            # branch: SP's first DMA starts at t=0 instead of t=50.
            for b, t0, t1, sem, eng in loads:
                nc.sync.dma_start(
                    xt[:, t0:t1], x[b, :, t0 - (b - 2) * T : t1 - (b - 2) * T]
                ).then_inc(sem, 16)

            block = e(nc.Block())

            @block.sync
            def _(sync):
                # Store B: items 5-7 (c5/c6/c7 bump c67_sem).
                sync.dma_start(
                    y[5:8, :, :].rearrange("b c f -> c b f"),
                    yt[:, 5 * F : 8 * F].rearrange("c (b f) -> c b f", f=F),
                )._wait_ge(c67_sem, 3).then_inc(sb_sem, 16)
                sync.wait_ge(sa_sem, 16)
                sync.wait_ge(sb_sem, 16)

            @block.vector
            def _(vector):
                vector.memset(ones_row[:, :], 1.0).then_inc(dve_sem, 1)
                first = True
                for key, src_t, a0, a1, out16, wsem in dve_reduces:
                    if wsem is None:
                        wsem = load_sems[key]
                    if out16:
                        with nc.allow_low_precision(
                            "bf16 sums feed bf16 matmuls; ~0.4% rounding"
                        ):
                            col = red_out16[key]
                            red = vector.reduce_sum(
                                out=sums16[:, col : col + 1],
                                in_=src_t[:, a0:a1],
                                axis=mybir.AxisListType.X,
                            )
                    else:
                        col = red_out32[key]
                        red = vector.reduce_sum(
                            out=sums[:, col : col + 1],
                            in_=src_t[:, a0:a1],
                            axis=mybir.AxisListType.X,
                        )
                    red._wait_ge(wsem, 16).then_inc(dve_sem, 1)
                    if first:
                        first = False
                        # s1 = (1+sigma)/T right after the first reduce.
                        vector.tensor_scalar(
                            out=s1[:, :],
                            in0=sg[:, :],
                            scalar1=1.0 / T,
                            scalar2=1.0 / T,
                            op0=mybir.AluOpType.mult,
                            op1=mybir.AluOpType.add,
                        )._wait_ge(sig_sem, 16).then_inc(s1_sem, 1)
                # Items 6 and 7's PSUM -> SBUF copies (GPSIMD may not
                # touch PSUM; DVE is free after its last reduce; ACT is
                # backlogged with item 6's accum-reduce and c1/c5).
                vector.tensor_copy(
                    yt[:, 6 * F : 7 * F], pts[6][:, :]
                )._wait_ge(pe_sem, MM_BY_ITEM[6]).then_inc(c67_sem, 1)
                vector.tensor_copy(
                    yt[:, 7 * F : 8 * F], pts[7][:, :]
                )._wait_ge(pe_sem, MM7_DONE).then_inc(c67_sem, 1)

            @block.gpsimd
            def _(gpsimd):
                # SWDGE loads: sigma first (tiny; its early semaphore
                # unblocks the s1 -> ident chain), then the casting pair
                # load of items 0/1 (fp32 -> bf16: half the DMA bytes).
                gpsimd.dma_start(sg[:, :], sig[:, :]).then_inc(sig_sem, 16)
                gpsimd.dma_start(
                    xt16[:, :].rearrange("p (b t) -> p b t", b=2),
                    x[0:2, :, :].rearrange("b p t -> p b t"),
                ).then_inc(c01_sem, 16)
                # ident_s = diag((1+sigma)/T): select between a broadcast
                # of scale_col and 0.0.
                gpsimd.affine_select(
                    out=ident_s[:, :],
                    in_=scale_col[:, :].broadcast_to((128, 128)),
                    compare_op=mybir.AluOpType.is_equal,
                    fill=0.0,
                    base=0,
                    pattern=[[-1, 128]],
                    channel_multiplier=1,
                )._wait_ge(act_sem, 1).then_inc(pool_sem, 1)
                gpsimd.tensor_copy(ident16[:, :], ident_s[:, :])
                # Store A (items 0-4) goes out via SWDGE: its descriptor
                # generation runs on the otherwise-idle Pool engine, so
                # store B has the HWDGE path entirely to itself on the
                # critical tail.
                gpsimd.dma_start(
                    y[0:5, :, :].rearrange("b c f -> c b f"),
                    yt[:, 0 : 5 * F].rearrange("c (b f) -> c b f", f=F),
                )._wait_ge(ca_sem, 5).then_inc(sa_sem, 16)

            @block.scalar
            def _(scalar):
                # scale_col = (1+sigma)/T on all partitions, via psc.
                scalar.activation(
                    out=scale_col[:, :],
                    in_=psc[:, :],
                    func=mybir.ActivationFunctionType.Copy,
                )._wait_ge(pe_sem, 1).then_inc(act_sem, 1)
                # Copies c2, c3, c4 (early, reduce-paced), then c0, then
                # item 6's accum-reduce, then c1 and c5 as their matmuls
                # land.  c6/c7 run on DVE.
                for bidx in (2, 3, 4):
                    scalar.activation(
                        out=yt[:, bidx * F : (bidx + 1) * F],
                        in_=pts[bidx][:, :],
                        func=mybir.ActivationFunctionType.Copy,
                    )._wait_ge(pe_sem, MM_BY_ITEM[bidx]).then_inc(ca_sem, 1)
                scalar.activation(
                    out=yt[:, 0 * F : 1 * F],
                    in_=pts[0][:, :],
                    func=mybir.ActivationFunctionType.Copy,
                )._wait_ge(pe_sem, MM_BY_ITEM[0]).then_inc(ca_sem, 1)
                # Item 6's reduce: activation+accumulate into sums col 6.
                scalar.activation(
                    out=dump[:, 0:T],
                    in_=xt[:, 4 * T : 5 * T],
                    func=mybir.ActivationFunctionType.Copy,
                    accum_out=sums[:, 6:7],
                )._wait_ge(load_sems[6], 16).then_inc(act_red_sem, 1)
                scalar.activation(
                    out=yt[:, 1 * F : 2 * F],
                    in_=pts[1][:, :],
                    func=mybir.ActivationFunctionType.Copy,
                )._wait_ge(pe_sem, MM_BY_ITEM[1]).then_inc(ca_sem, 1)
                scalar.activation(
                    out=yt[:, 5 * F : 6 * F],
                    in_=pts[5][:, :],
                    func=mybir.ActivationFunctionType.Copy,
                )._wait_ge(pe_sem, MM_BY_ITEM[5]).then_inc(c67_sem, 1)

            @block.tensor
            def _(tensor):
                # Standalone waits keep PE.SEQ occupied between matmuls,
                # pinning pe_busy_start so the p-state ramps to full speed.
                # psc[p, 0] = (1+sigma)/T on every partition (K=1 matmul).
                tensor.wait_ge(dve_sem, 1)  # ones_row
                tensor.wait_ge(s1_sem, 1)
                tensor.matmul(
                    psc[:, :], ones_row[:, :], s1[:, :], start=True, stop=True
                ).then_inc(pe_sem, 1)
                tensor.wait_ge(pool_sem, 1)  # ident_s ready
                red_key = {
                    ("b16", 0): "01a",
                    ("b16", 1): "01b",
                    ("b16", 2): "7a",
                    ("b16", 3): "7b",
                    ("f32", 2): 2,
                    ("f32", 3): 3,
                    ("f32", 4): 4,
                    ("f32", 5): 5,
                }
                for kind, b, col, is_start, is_stop in mm_order:
                    if kind == "mm6":
                        tensor.wait_ge(act_red_sem, 1)
                        lhsT = sums[:, 6:7].broadcast_to((128, C))
                        rhs = ident_s[:, :]
                    else:
                        tensor.wait_ge(dve_sem, dve_ms[red_key[(kind, col)]])
                        if kind == "b16":
                            lhsT = sums16[:, col : col + 1].broadcast_to(
                                (128, C)
                            )
                            rhs = ident16[:, :]
                        else:
                            lhsT = sums[:, col : col + 1].broadcast_to(
                                (128, C)
                            )
                            rhs = ident_s[:, :]
                    tensor.matmul(
                        pts[b][:, :],
                        lhsT,
                        rhs,
                        start=is_start,
                        stop=is_stop,
                    ).then_inc(pe_sem, 1)

    finally:
        bass.BassEitherVectorEngine.memset = _orig_memset
        bass.Bass.all_engine_barrier = _orig_barrier
        bass.BassEngine.preamble = _orig_preamble

    _NC_CACHE = nc
    return nc


def run_spmd(inputs_arr: np.ndarray, sigma_arr: np.ndarray, trace: bool = False):
    """Shard over batch, run on 8 cores, gather. Returns (out, results_obj)."""
    from concourse import bass_utils

    nc = _build_bass()

    x_full = np.ascontiguousarray(np.asarray(inputs_arr, dtype=np.float32))
    assert x_full.shape == (B, F, T), x_full.shape
    sig = np.asarray(sigma_arr, dtype=np.float32).reshape(1, 1)

    in_maps = [
        {"x": x_full[k * BPC : (k + 1) * BPC], "sig": sig} for k in range(N_CORES)
    ]
    res = bass_utils.run_bass_kernel_spmd(
        nc, in_maps, core_ids=list(range(N_CORES)), trace=trace
    )
    out = np.concatenate([r["y"] for r in res.results], axis=0)
    return out, res


def kernel(**inputs) -> np.ndarray:
    out, _ = run_spmd(inputs["inputs"], inputs["sigma"])
    return out


# revision 38
# speedup vs baseline: 1.0157x; 1.0047x over previous
"""Trainium2 Bass kernel for nn_CSAModule_47768626266174.

Final variant: items 0/1 arrive through ONE casting SWDGE load
(fp32->bf16, descriptors generated on the otherwise-idle GPSIMD
engine; the DMA engines move half the bytes, shortening the load
stream ~720 ns); sigma loads through a SWDGE descriptor generated
first.  Store A (items 0-4) also goes out via SWDGE so store B
(items 5-7, the only store on the critical tail) has the HWDGE
launch path entirely to itself.

Mathematical structure of the reference:

    S    = softmax(attn, axis=-1)                # [C, T, T]
    out  = base + sigma * einsum('bft,ct->bcf', inputs, S.mean(axis=-1))
    base = inputs.mean(-1)[:, None, :]

``S.mean(axis=-1)`` averages over the *same* axis the softmax normalizes,
so it is exactly 1/T for every (c, t) — independent of the attention
contents, the conv weights, and the labels.  Hence

    out[b, c, f] = (1 + sigma) * mean_t inputs[b, f, t]

for every class c, so the kernel only needs to read ``inputs`` once,
reduce over T, scale by (1 + sigma)/T, and broadcast over the class dim.

Sharding: data-parallel over batch B — each of the 8 cores reduces its
8-item chunk; no collectives.  Output chunks are concatenated on host.

Per-core dataflow.  The critical path is the serialized DMA data stream
(~2.1 MB at 360 GB/s), then the tail chain of the LAST-arriving chunk:
dma-sem (900) -> reduce -> matmul -> PSUM->SBUF copy -> store launch
(HWDGE 625 + DGE 650) -> transfer -> dma-sem (900).  Design notes:

  * SP's 5-instruction register preamble is skipped (-250 ns stream
    start; SP only issues DMAs / waits, which never read those regs).
  * 9 load DMAs (HWDGE is 625 ns per DMA; a 10th would stall the
    stream): items 0-6 whole, item 7 in halves so the last chunk's
    reduce is half-length.  Each load has its own semaphore (dynamic
    queues complete out of order).
  * DVE reduces items 0-5 and 7a/7b; item 6 reduces on ACT via
    activation+accumulate, so DVE is FREE exactly when 7a/7b's
    semaphores fire and the last reduces run semaphore-paced.
  * Item 7's sums are written in bf16 and its two matmuls use a bf16
    scaled-identity: 1 PE cycle/row instead of fp32's 4 (53 vs 213 ns).
    ~0.4% rounding on 1/8 of the output, vs a 2e-2 rel-err budget.
  * PE matmul order: items 0-5, then 7a(start), 6, 7b(stop).  mm7a's
    input is ready before mm6's (ACT's accum-read chain is slow) and
    mm7b's reduce lands last, so this keeps PE busy without delaying
    mm7b.  pts[6]/pts[7] are separate PSUM banks so the interleaved
    accumulation groups don't interact.  PE uses STANDALONE wait_ge
    (not waits attached to the matmul): the busy sequencer pins
    pe_busy_start so the PE p-state ramps to full speed (213 ns/mm)
    instead of restarting cold (607 ns/mm) at every matmul.
  * Copies: items 0-5 and 6 on ACT into yt; item 7 on DVE (GPSIMD may
    not touch PSUM; DVE is free after its last reduce).
  * Stores: A = y[0:5] via GPSIMD SWDGE (desc-gen off the critical
    path), B = y[5:8] via SP HWDGE, gated on the item-5/6/7 copies.
    SWDGE prepare/trigger stores would launch ~1.2 us faster after
    the data is ready, but this walrus build cannot encode
    InstTriggerDma ("ISA wrong length").
  * The Bass start barrier, its dead const-tile memsets, and the
    Block-exit all-engine barrier are skipped; SP's final waits on the
    three store semaphores keep the program alive until y is in HBM.
"""

from contextlib import ExitStack

import numpy as np

B, F, T, C = 64, 128, 512, 10
N_CORES = 8
BPC = B // N_CORES  # batch items per core
H = T // 2

_NC_CACHE = None


def _build_bass():
    """Build the per-core Bass module (SPMD: same program on all cores)."""
    global _NC_CACHE
    if _NC_CACHE is not None:
        return _NC_CACHE

    import concourse.bass as bass
    import concourse.mybir as mybir

    fp32 = mybir.dt.float32
    bf16 = mybir.dt.bfloat16

    _orig_memset = bass.BassEitherVectorEngine.memset

    def _memset_skip_dead_consts(self, ap, constant):
        tensor = getattr(ap, "tensor", None)
        if tensor is not None and getattr(tensor, "name", "").startswith(
            "const-"
        ):
            return None
        return _orig_memset(self, ap, constant)

    _orig_barrier = bass.Bass.all_engine_barrier

    def _skip_barrier(self, *, sem_only: bool = False):
        return None

    _orig_preamble = bass.BassEngine.preamble

    def _preamble_skip_sp(self):
        if self.engine == mybir.EngineType.SP:
            return None
        return _orig_preamble(self)

    bass.BassEitherVectorEngine.memset = _memset_skip_dead_consts
    bass.Bass.all_engine_barrier = _skip_barrier
    bass.BassEngine.preamble = _preamble_skip_sp
    try:
        nc = bass.Bass()

        x = nc.dram_tensor("x", [BPC, F, T], fp32, kind="ExternalInput")
        sig = nc.dram_tensor("sig", [1, 1], fp32, kind="ExternalInput")
        y = nc.dram_tensor("y", [BPC, C, F], fp32, kind="ExternalOutput")

        with ExitStack() as ctx:
            e = ctx.enter_context
            # xt holds items 2..7 (fp32): item b at cols (b-2)*T.
            # Items 0/1 land as bf16 via one casting SWDGE load.
            xt = e(nc.sbuf_tensor("xt", [128, 6 * T], fp32))
            xt16 = e(nc.sbuf_tensor("xt16", [128, 2 * T], bf16))
            # sums: fp32 cols for items 2-6; bf16 cols 0,1=items 0,1 and
            # 6,7=item 7's halves.
            sums = e(nc.sbuf_tensor("sums", [128, 8], fp32))
            sums16 = e(nc.sbuf_tensor("sums16", [128, 8], bf16))
            ident_s = e(nc.sbuf_tensor("ident_s", [128, 128], fp32))
            ident16 = e(nc.sbuf_tensor("ident16", [128, 128], bf16))
            sg = e(nc.sbuf_tensor("sg", [1, 1], fp32))
            s1 = e(nc.sbuf_tensor("s1", [1, 1], fp32))
            ones_row = e(nc.sbuf_tensor("ones_row", [1, 128], fp32))
            scale_col = e(nc.sbuf_tensor("scale_col", [128, 1], fp32))
            yt = e(nc.sbuf_tensor("yt", [C, BPC * F], fp32))
            dump = e(nc.sbuf_tensor("dump", [128, T], fp32))
            # psc is allocated and immediately freed: its bank is reused by
            # pts[0].  Safe because mm0 (the first pts[0] write) waits for
            # ident_s, which waits for the scale_col copy — psc's last read.
            psc_cm = nc.psum_tensor("psc", [128, 1], fp32)
            psc = psc_cm.__enter__()
            psc_cm.__exit__(None, None, None)
            pts = [
                e(nc.psum_tensor(f"pt{b}", [C, 128], fp32))
                for b in range(BPC)
            ]

            c01_sem = e(nc.semaphore("xc01"))
            # HWDGE loads (SP): (item, xt-t0, t1, sem, engine); item b at
            # xt cols (b-2)*T.
            loads = []
            for b in range(2, 6):
                loads.append(
                    (b, (b - 2) * T, (b - 1) * T, e(nc.semaphore(f"x{b}")), "dve")
                )
            loads.append((6, 4 * T, 5 * T, e(nc.semaphore("x6")), "act"))
            x7a_sem = e(nc.semaphore("x7a"))
            x7b_sem = e(nc.semaphore("x7b"))
            loads.append((7, 5 * T, 5 * T + H, x7a_sem, "dve16"))
            loads.append((7, 5 * T + H, 6 * T, x7b_sem, "dve16"))

            sig_sem = e(nc.semaphore("sig_sem"))
            s1_sem = e(nc.semaphore("s1_sem"))
            dve_sem = e(nc.semaphore("dve_sem"))
            pool_sem = e(nc.semaphore("pool_sem"))
            pe_sem = e(nc.semaphore("pe_sem"))
            act_sem = e(nc.semaphore("act_sem"))
            act_red_sem = e(nc.semaphore("act_red_sem"))
            c67_sem = e(nc.semaphore("c67_sem"))
            ca_sem = e(nc.semaphore("ca_sem"))
            sa_sem = e(nc.semaphore("sa_sem"))
            sb_sem = e(nc.semaphore("sb_sem"))
            sc_sem = e(nc.semaphore("sc_sem"))

            # DVE reduce order (by semaphore time): r2, r3, r4, then the
            # cast pair r0/r1, r5, then item 7's halves.  Reduce spec:
            # (key, src, s0, s1col, out16?, waitsem).  sums16 cols: 0,1 =
            # items 0,1; 2,3 = item 7's halves.  sums (fp32) cols 2..5 =
            # items 2..5 (item 6 accumulates into col 6 on ACT).
            dve_reduces = [
                (2, xt, 0 * T, 1 * T, False, None),
                (3, xt, 1 * T, 2 * T, False, None),
                (4, xt, 2 * T, 3 * T, False, None),
                ("01a", xt16, 0 * T, 1 * T, True, c01_sem),
                ("01b", xt16, 1 * T, 2 * T, True, c01_sem),
                (5, xt, 3 * T, 4 * T, False, None),
                ("7a", xt, 5 * T, 5 * T + H, True, x7a_sem),
                ("7b", xt, 5 * T + H, 6 * T, True, x7b_sem),
            ]
            load_sems = {b: sem for b, t0, t1, sem, eng in loads}
            red_out16 = {"01a": 0, "01b": 1, "7a": 2, "7b": 3}
            red_out32 = {2: 2, 3: 3, 4: 4, 5: 5}
            dve_ms = {}
            n = 1
            for entry in dve_reduces:
                n += 1
                dve_ms[entry[0]] = n
            # pe_sem: 1 = psc, then the matmuls in PE program order.
            # ("f32", item, sums-col) / ("b16", item, sums16-col, start,
            # stop).  Item 7 accumulates in pts[7]; mm6 interleaves
            # between mm7a and mm7b (different PSUM banks).
            mm_order = [
                ("f32", 2, 2, True, True),
                ("f32", 3, 3, True, True),
                ("f32", 4, 4, True, True),
                ("b16", 0, 0, True, True),
                ("b16", 1, 1, True, True),
                ("f32", 5, 5, True, True),
                ("b16", 7, 2, True, False),
                ("mm6", 6, 6, True, True),
                ("b16", 7, 3, False, True),
            ]
            mm_ms = {}
            for i, ent in enumerate(mm_order):
                mm_ms[(ent[0], ent[2])] = i + 2
            MM_BY_ITEM = {
                0: mm_ms[("b16", 0)],
                1: mm_ms[("b16", 1)],
                2: mm_ms[("f32", 2)],
                3: mm_ms[("f32", 3)],
                4: mm_ms[("f32", 4)],
                5: mm_ms[("f32", 5)],
                6: mm_ms[("mm6", 6)],
                7: mm_ms[("b16", 3)],
            }
            MM7_DONE = MM_BY_ITEM[7]

            # Loads issue in the main body, before the Block's entry
            # branch: SP's first DMA starts at t=0 instead of t=50.
            for b, t0, t1, sem, eng in loads:
                nc.sync.dma_start(
                    xt[:, t0:t1], x[b, :, t0 - (b - 2) * T : t1 - (b - 2) * T]
                ).then_inc(sem, 16)

            # SP's stores and final waits stay in the main body: with no
            # @block.sync section, SP never pays the Block-entry branch and
            # its last instruction is the final wait itself.
            nc.sync.dma_start(
                y[5:8, :, :].rearrange("b c f -> c b f"),
                yt[:, 5 * F : 8 * F].rearrange("c (b f) -> c b f", f=F),
            )._wait_ge(c67_sem, 3).then_inc(sb_sem, 16)
            nc.sync.wait_ge(sa_sem, 16)
            nc.sync.wait_ge(sb_sem, 16)

            block = e(nc.Block())

            @block.vector
            def _(vector):
                vector.memset(ones_row[:, :], 1.0).then_inc(dve_sem, 1)
                first = True
                for key, src_t, a0, a1, out16, wsem in dve_reduces:
                    if wsem is None:
                        wsem = load_sems[key]
                    if out16:
                        with nc.allow_low_precision(
                            "bf16 sums feed bf16 matmuls; ~0.4% rounding"
                        ):
                            col = red_out16[key]
                            red = vector.reduce_sum(
                                out=sums16[:, col : col + 1],
                                in_=src_t[:, a0:a1],
                                axis=mybir.AxisListType.X,
                            )
                    else:
                        col = red_out32[key]
                        red = vector.reduce_sum(
                            out=sums[:, col : col + 1],
                            in_=src_t[:, a0:a1],
                            axis=mybir.AxisListType.X,
                        )
                    red._wait_ge(wsem, 16).then_inc(dve_sem, 1)
                    if first:
                        first = False
                        # s1 = (1+sigma)/T right after the first reduce.
                        vector.tensor_scalar(
                            out=s1[:, :],
                            in0=sg[:, :],
                            scalar1=1.0 / T,
                            scalar2=1.0 / T,
                            op0=mybir.AluOpType.mult,
                            op1=mybir.AluOpType.add,
                        )._wait_ge(sig_sem, 16).then_inc(s1_sem, 1)
                # Items 6 and 7's PSUM -> SBUF copies (GPSIMD may not
                # touch PSUM; DVE is free after its last reduce; ACT is
                # backlogged with item 6's accum-reduce and c1/c5).
                vector.tensor_copy(
                    yt[:, 6 * F : 7 * F], pts[6][:, :]
                )._wait_ge(pe_sem, MM_BY_ITEM[6]).then_inc(c67_sem, 1)
                vector.tensor_copy(
                    yt[:, 7 * F : 8 * F], pts[7][:, :]
                )._wait_ge(pe_sem, MM7_DONE).then_inc(c67_sem, 1)

            @block.gpsimd
            def _(gpsimd):
                # SWDGE loads: sigma first (tiny; its early semaphore
                # unblocks the s1 -> ident chain), then the casting pair
                # load of items 0/1 (fp32 -> bf16: half the DMA bytes).
                gpsimd.dma_start(sg[:, :], sig[:, :]).then_inc(sig_sem, 16)
                gpsimd.dma_start(
                    xt16[:, :].rearrange("p (b t) -> p b t", b=2),
                    x[0:2, :, :].rearrange("b p t -> p b t"),
                ).then_inc(c01_sem, 16)
                # ident_s = diag((1+sigma)/T): select between a broadcast
                # of scale_col and 0.0.
                gpsimd.affine_select(
                    out=ident_s[:, :],
                    in_=scale_col[:, :].broadcast_to((128, 128)),
                    compare_op=mybir.AluOpType.is_equal,
                    fill=0.0,
                    base=0,
                    pattern=[[-1, 128]],
                    channel_multiplier=1,
                )._wait_ge(act_sem, 1).then_inc(pool_sem, 1)
                gpsimd.tensor_copy(ident16[:, :], ident_s[:, :])
                # Store A (items 0-4) goes out via SWDGE: its descriptor
                # generation runs on the otherwise-idle Pool engine, so
                # store B has the HWDGE path entirely to itself on the
                # critical tail.
                gpsimd.dma_start(
                    y[0:5, :, :].rearrange("b c f -> c b f"),
                    yt[:, 0 : 5 * F].rearrange("c (b f) -> c b f", f=F),
                )._wait_ge(ca_sem, 5).then_inc(sa_sem, 16)

            @block.scalar
            def _(scalar):
                # scale_col = (1+sigma)/T on all partitions, via psc.
                scalar.activation(
                    out=scale_col[:, :],
                    in_=psc[:, :],
                    func=mybir.ActivationFunctionType.Copy,
                )._wait_ge(pe_sem, 1).then_inc(act_sem, 1)
                # Copies c2, c3, c4 (early, reduce-paced), then c0, then
                # item 6's accum-reduce, then c1 and c5 as their matmuls
                # land.  c6/c7 run on DVE.
                for bidx in (2, 3, 4):
                    scalar.activation(
                        out=yt[:, bidx * F : (bidx + 1) * F],
                        in_=pts[bidx][:, :],
                        func=mybir.ActivationFunctionType.Copy,
                    )._wait_ge(pe_sem, MM_BY_ITEM[bidx]).then_inc(ca_sem, 1)
                scalar.activation(
                    out=yt[:, 0 * F : 1 * F],
                    in_=pts[0][:, :],
                    func=mybir.ActivationFunctionType.Copy,
                )._wait_ge(pe_sem, MM_BY_ITEM[0]).then_inc(ca_sem, 1)
                # Item 6's reduce: activation+accumulate into sums col 6.
                scalar.activation(
                    out=dump[:, 0:T],
                    in_=xt[:, 4 * T : 5 * T],
                    func=mybir.ActivationFunctionType.Copy,
                    accum_out=sums[:, 6:7],
                )._wait_ge(load_sems[6], 16).then_inc(act_red_sem, 1)
                scalar.activation(
                    out=yt[:, 1 * F : 2 * F],
                    in_=pts[1][:, :],
                    func=mybir.ActivationFunctionType.Copy,
                )._wait_ge(pe_sem, MM_BY_ITEM[1]).then_inc(ca_sem, 1)
                scalar.activation(
                    out=yt[:, 5 * F : 6 * F],
                    in_=pts[5][:, :],
                    func=mybir.ActivationFunctionType.Copy,
                )._wait_ge(pe_sem, MM_BY_ITEM[5]).then_inc(c67_sem, 1)

            @block.tensor
            def _(tensor):
                # Standalone waits keep PE.SEQ occupied between matmuls,
                # pinning pe_busy_start so the p-state ramps to full speed.
                # psc[p, 0] = (1+sigma)/T on every partition (K=1 matmul).
                tensor.wait_ge(dve_sem, 1)  # ones_row
                tensor.wait_ge(s1_sem, 1)
                tensor.matmul(
                    psc[:, :], ones_row[:, :], s1[:, :], start=True, stop=True
                ).then_inc(pe_sem, 1)
                tensor.wait_ge(pool_sem, 1)  # ident_s ready
                red_key = {
                    ("b16", 0): "01a",
                    ("b16", 1): "01b",
                    ("b16", 2): "7a",
                    ("b16", 3): "7b",
                    ("f32", 2): 2,
                    ("f32", 3): 3,
                    ("f32", 4): 4,
                    ("f32", 5): 5,
                }
                for kind, b, col, is_start, is_stop in mm_order:
                    if kind == "mm6":
                        tensor.wait_ge(act_red_sem, 1)
                        lhsT = sums[:, 6:7].broadcast_to((128, C))
                        rhs = ident_s[:, :]
                    else:
                        tensor.wait_ge(dve_sem, dve_ms[red_key[(kind, col)]])
                        if kind == "b16":
                            lhsT = sums16[:, col : col + 1].broadcast_to(
                                (128, C)
                            )
                            rhs = ident16[:, :]
                        else:
                            lhsT = sums[:, col : col + 1].broadcast_to(
                                (128, C)
                            )
                            rhs = ident_s[:, :]
                    tensor.matmul(
                        pts[b][:, :],
                        lhsT,
                        rhs,
                        start=is_start,
                        stop=is_stop,
                    ).then_inc(pe_sem, 1)

    finally:
        bass.BassEitherVectorEngine.memset = _orig_memset
        bass.Bass.all_engine_barrier = _orig_barrier
        bass.BassEngine.preamble = _orig_preamble

    _NC_CACHE = nc
    return nc


def run_spmd(inputs_arr: np.ndarray, sigma_arr: np.ndarray, trace: bool = False):
    """Shard over batch, run on 8 cores, gather. Returns (out, results_obj)."""
    from concourse import bass_utils

    nc = _build_bass()

    x_full = np.ascontiguousarray(np.asarray(inputs_arr, dtype=np.float32))
    assert x_full.shape == (B, F, T), x_full.shape
    sig = np.asarray(sigma_arr, dtype=np.float32).reshape(1, 1)

    in_maps = [
        {"x": x_full[k * BPC : (k + 1) * BPC], "sig": sig} for k in range(N_CORES)
    ]
    res = bass_utils.run_bass_kernel_spmd(
        nc, in_maps, core_ids=list(range(N_CORES)), trace=trace
    )
    out = np.concatenate([r["y"] for r in res.results], axis=0)
    return out, res


def kernel(**inputs) -> np.ndarray:
    out, _ = run_spmd(inputs["inputs"], inputs["sigma"])
    return out
